# revision 8
# baseline (speedup 1.0000x reference)
"""Trainium2 Bass kernel for nn_Classifier_6717328851414 (dense x-major, y-half pipelined, DVE c1-inject).

DEQ-style classifier:
  K iterations of  z <- 0.5*z + 0.5*lrelu(conv2(lrelu(conv1(cat(z, img)))))
  conv1: 8->6 ch 5x5 pad 2; conv2: 6->5 ch 5x5 pad 2; 32x32 images; then a
  5->10 channel 32x32 valid "head" conv producing logits (N,10,1,1).

The fixed point z* = f(z*) is independent of the damping factor, so the
kernel iterates UNDAMPED (alpha=1: z <- f(z)), which contracts at ~0.6/iter
vs the reference's 0.78 (alpha=0.5). ITERS=13 reaches 2.0e-3 of the 150-iter
reference (tolerance 2e-2, ~10x margin), and the z-update reduces to the
ScalarE activation writing the z slab directly (no DVE damping op).

Strategy: pure data parallel over batch N=512 -> 64 images per core.

Per-core layout (fp16 activations, fp32 PSUM):
  Channels padded to 6 slots (z: 5 real + 1 zero; h1: 6 real).
  Two x-chunks with 4-column overlap (halo), partition p = (x - x0)*6 + c:
    slab A: x in [-2, 18)   (x0=-2,  120 partitions used of 128)
    slab B: x in [14, 34)   (x0=14,  120 partitions)
  so the same x maps to pA = pB + 96 (32-aligned partition shift -> legal
  cross-partition DVE halo copies).
  Free dim: (y_padded, n) = 36*64 = 2304; y rows 0,1,34,35 stay zero.

Each conv chunk is ONE dense-K matmul group: K = full 120-partition x-window,
M = (x',co) output columns (108 for A, 120 for B), accumulated over the 5 ky
taps by shifting the moving AP along y, one matmul per (quarter, ky).
Bias + leaky-relu are fused in one ScalarE activation (Lrelu, alpha=0.01);
the damping z <- 0.5 z + v is one DVE scalar_tensor_tensor. The constant
image contribution to conv1 (c1 = conv(img) over channels 5:8, precomputed
on the host) is injected by a DVE
scalar_tensor_tensor (PSUM read -> SBUF staging; the DVE has no PSUM write
port) instead of a PE identity matmul, and kernel taps whose input rows are
all padding are clipped (start=True rides the always-full ky=2 tap), leaving
80 matmuls / ~39.4k streamed columns per iteration (~17.5 us/iter vs the
banded baseline's ~34.7 us/iter).
"""

import numpy as np

import concourse.bass as bass
import concourse.mybir as mybir
import concourse.tile as tile
from concourse.vector_clock import ScopedClock, VectorClock

ITERS = 13
SLOPE = 0.01
NCORES = 8
NTOT = 512
NPER = NTOT // NCORES  # 64
YP = 36
FREE = YP * NPER  # 2304
CH = 6
MA = 108  # chunk A output cols: x' in [-2,16)
MB = 120  # chunk B output cols: x' in [14,34)
KW = 120  # moving K: 20 x-cols * 6
F32 = mybir.dt.float32
F16 = mybir.dt.float16
AF = mybir.ActivationFunctionType
OP = mybir.AluOpType


def _patched_drain_and_barrier(self, tick_clock, wait_clock):
    # Workaround: walrus rejects >2 sync waits on one instruction; split the
    # final drain's waits across one SP nop per logical processor.
    gc = tick_clock.global_clock
    n = len(gc)
    for p in range(n):
        if gc[p] == 0:
            continue
        vc = VectorClock([gc[q] if q == p else 0 for q in range(n)])
        nop = self.nc.sync.nop(nofuse=True)
        wait_clock.add_sem_waits(nop.ins, ScopedClock({None: vc}))
    self.nc.sync.drain()
    self.nc.all_engine_barrier()
    assert self.sems is not None
    popped = self.nc._tile_sem_poison_stack.pop()
    assert popped is self._sem_poison
    self.nc.clear_and_free_semaphores(list(self.sems.allocated().values()))
    self.nc.all_engine_barrier()


tile.TileContext._drain_and_barrier = _patched_drain_and_barrier


def _split_excess_waits(nc, limit=1):
    """Hoist excess sync waits onto same-engine NoOps (walrus limit)."""
    for bb in nc.main_func.blocks:
        out = []
        changed = False
        for ins in bb.instructions:
            lim = limit
            si = ins.sync_info
            waits = list(si.on_wait) if (si is not None and si.on_wait) else []
            if len(waits) > lim:
                extra, keep = waits[:-lim], waits[-lim:]
                for i0 in range(0, len(extra), limit):
                    nop = mybir.InstNoOp(
                        name=nc.get_next_instruction_name(),
                        engine=ins.engine,
                        ins=[],
                        outs=[],
                        sync_info=mybir.SyncInfo(
                            on_wait=extra[i0 : i0 + limit], on_update=[]
                        ),
                    )
                    out.append(nop)
                si.on_wait = keep
                changed = True
            out.append(ins)
        if changed:
            bb.instructions = out


def build_nc(iters=ITERS, unroll=4):
    nc = bass.Bass()

    w1sa_p = nc.declare_dram_parameter("w1sa", [KW, 5 * MA], F16, isOutput=False)
    w1sb_p = nc.declare_dram_parameter("w1sb", [KW, 5 * MB], F16, isOutput=False)
    w2sa_p = nc.declare_dram_parameter("w2sa", [KW, 5 * MA], F16, isOutput=False)
    w2sb_p = nc.declare_dram_parameter("w2sb", [KW, 5 * MB], F16, isOutput=False)
    wha_p = nc.declare_dram_parameter("wha", [KW, 320], F16, isOutput=False)
    whb_p = nc.declare_dram_parameter("whb", [KW, 320], F16, isOutput=False)
    bias_p = nc.declare_dram_parameter("bias", [128, 8], F32, isOutput=False)
    c1a_p = nc.declare_dram_parameter("c1a", [MA, FREE], F16, isOutput=False)
    c1b_p = nc.declare_dram_parameter("c1b", [MB, FREE], F16, isOutput=False)
    out_p = nc.declare_dram_parameter("out", [10, NPER], F32, isOutput=True)

    with tile.TileContext(nc) as tc:
        with (
            tc.tile_pool(name="const", bufs=1) as cpool,
            tc.tile_pool(name="state", bufs=1) as spool,
            tc.tile_pool(name="psum", bufs=8, space="PSUM") as ppool,
            tc.tile_pool(name="stage", bufs=4) as vpool,
        ):
            w1sa = cpool.tile([KW, 5 * MA], F16, tag="w1sa")
            w1sb = cpool.tile([KW, 5 * MB], F16, tag="w1sb")
            w2sa = cpool.tile([KW, 5 * MA], F16, tag="w2sa")
            w2sb = cpool.tile([KW, 5 * MB], F16, tag="w2sb")
            wha = cpool.tile([KW, 320], F16, tag="wha")
            whb = cpool.tile([KW, 320], F16, tag="whb")
            bias = cpool.tile([128, 8], F32, tag="bias")
            c1a = cpool.tile([MA, FREE], F16, tag="c1a")
            c1b = cpool.tile([MB, FREE], F16, tag="c1b")
            for t, p in (
                (w1sa, w1sa_p), (w1sb, w1sb_p), (w2sa, w2sa_p), (w2sb, w2sb_p),
                (wha, wha_p), (whb, whb_p), (bias, bias_p),
                (c1a, c1a_p), (c1b, c1b_p),
            ):
                nc.sync.dma_start(t[:], p[:])

            # y-half-split state: half 0 = y in [-2,18) (rows y+2), half 1 =
            # y in [14,34) (rows y-14); 20 padded rows each, 4-row overlap.
            # Quarters 0,1 read/write half 0; quarters 2,3 half 1 -- a
            # quarter's 5-tap y-window never crosses its half. Halo closes
            # for half 0 are issued between the q2 and q3 matmul groups (they
            # depend only on q0..q2), so the next stage's q0/q1 matmuls are
            # ready the moment the PE finishes q3 -- no stage-transition
            # stalls.
            HF = 20 * NPER
            zs = {k: spool.tile([128, HF], F16, tag=f"z{k}", name=f"z{k}")
                  for k in ("A0", "A1", "B0", "B1")}
            h1s = {k: spool.tile([128, HF], F16, tag=f"h1{k}", name=f"h1{k}")
                   for k in ("A0", "A1", "B0", "B1")}
            for t in list(zs.values()) + list(h1s.values()):
                nc.gpsimd.memset(t[:], 0.0)
            R = NPER

            def half0_close(ts):
                # Close half 0 of all four slabs using only q0..q2 content.
                # Regions copied while transiently stale are re-fixed by a
                # later step (s5 fixes A0 rows 18,19; s6a/s6b fix B0's).
                nc.vector.tensor_copy(ts["A1"][:, 0 : 2 * R], ts["A0"][:, 16 * R : 18 * R])          # s1
                nc.vector.tensor_copy(ts["B0"][0:12, :], ts["A0"][96:108, :])                        # s2
                nc.vector.tensor_copy(ts["A0"][:, 18 * R : 20 * R], ts["A1"][:, 2 * R : 4 * R])      # s5
                nc.vector.tensor_copy(ts["B0"][:, 18 * R : 20 * R], ts["B1"][:, 2 * R : 4 * R])       # s6b (partitions 0:12 transiently stale)
                nc.vector.tensor_copy(ts["B0"][0:12, 18 * R : 20 * R], ts["A1"][96:108, 2 * R : 4 * R])  # s6a fixes them
                nc.vector.tensor_copy(ts["A0"][96:128, :], ts["B0"][0:32, :])                        # s7

            def half1_close(ts):
                nc.vector.tensor_copy(ts["B1"][0:12, :], ts["A1"][96:108, :])                        # s3
                nc.vector.tensor_copy(ts["B1"][:, 0 : 2 * R], ts["B0"][:, 16 * R : 18 * R])          # s4
                nc.vector.tensor_copy(ts["A1"][96:128, :], ts["B1"][0:32, :])                        # s8

            def jrng(q, ky):
                # output rows j in [0,8) whose input row 8q+j+ky-2 is real;
                # clipped taps only drop all-zero pad-row contributions.
                return max(0, 2 - ky - 8 * q), min(8, 34 - ky - 8 * q)

            KYS = (2, 0, 1, 3, 4)  # full-coverage tap first (carries start=True)

            def conv1_group(q):
                h = "01"[q // 2]
                r = 8 * (q % 2)
                for M, c1t, w1t, bcol, ab in (
                    (MB, c1b, w1sb, 1, "B"),
                    (MA, c1a, w1sa, 0, "A"),
                ):
                    zt, h1t = zs[ab + h], h1s[ab + h]
                    ps = ppool.tile([M, 512], F32, tag="ps")
                    for i, ky in enumerate(KYS):
                        jlo, jhi = jrng(q, ky)
                        nc.tensor.matmul(
                            ps[:, jlo * R : jhi * R],
                            w1t[:, ky * M : (ky + 1) * M],
                            zt[0:KW, (r + ky + jlo) * R : (r + ky + jhi) * R],
                            start=(i == 0), stop=(i == 4),
                        )
                    co = (8 * q + 2) * R
                    # inject the constant image term on the DVE (PSUM read,
                    # SBUF write -- the DVE has no PSUM write port), then
                    # bias+lrelu on ScalarE from SBUF.
                    u = vpool.tile([M, 512], F16, tag="u")
                    nc.vector.scalar_tensor_tensor(
                        u[:], ps[:], 1.0, c1t[:, co : co + 512], OP.mult, OP.add
                    )
                    nc.scalar.activation(
                        h1t[0:M, (r + 2) * R : (r + 2) * R + 512], u[:], AF.Lrelu,
                        bias=bias[0:M, bcol : bcol + 1], scale=1.0, alpha=SLOPE,
                    )

            def conv2_group(q):
                h = "01"[q // 2]
                r = 8 * (q % 2)
                for M, w2t, bcol, ab in (
                    (MB, w2sb, 3, "B"),
                    (MA, w2sa, 2, "A"),
                ):
                    h1t, zt = h1s[ab + h], zs[ab + h]
                    ps = ppool.tile([M, 512], F32, tag="ps")
                    for i, ky in enumerate(KYS):
                        jlo, jhi = jrng(q, ky)
                        nc.tensor.matmul(
                            ps[:, jlo * R : jhi * R],
                            w2t[:, ky * M : (ky + 1) * M],
                            h1t[0:KW, (r + ky + jlo) * R : (r + ky + jhi) * R],
                            start=(i == 0), stop=(i == 4),
                        )
                    nc.scalar.activation(
                        zt[0:M, (r + 2) * R : (r + 2) * R + 512], ps[:], AF.Lrelu,
                        bias=bias[0:M, bcol : bcol + 1], scale=1.0, alpha=SLOPE,
                    )

            def one_iter():
                for q in (0, 1, 2):
                    conv1_group(q)
                half0_close(h1s)
                conv1_group(3)
                half1_close(h1s)
                for q in (0, 1, 2):
                    conv2_group(q)
                half0_close(zs)
                conv2_group(3)
                half1_close(zs)

            trips, rem = divmod(iters, unroll)
            if trips > 0:
                with tc.For_i(0, trips, 1):
                    for _ in range(unroll):
                        one_iter()
            for _ in range(rem):
                one_iter()

            # ---- head: logits[k, n] = sum_{c,y,x} wh * z + bh
            psh = ppool.tile([10, NPER], F32, tag="ps")
            for y in range(32):
                h = "0" if y < 16 else "1"
                off = ((y + 2) if y < 16 else (y - 14)) * NPER
                nc.tensor.matmul(
                    psh[:], wha[:, y * 10 : (y + 1) * 10],
                    zs["A" + h][0:KW, off : off + NPER],
                    start=(y == 0), stop=False,
                )
                nc.tensor.matmul(
                    psh[:], whb[:, y * 10 : (y + 1) * 10],
                    zs["B" + h][0:KW, off : off + NPER],
                    start=False, stop=(y == 31),
                )
            out_sb = vpool.tile([10, NPER], F32, tag="osb")
            nc.scalar.activation(
                out_sb[:], psh[:], AF.Identity, bias=bias[0:10, 4:5], scale=1.0
            )
            nc.sync.dma_start(out_p[:], out_sb[:])

    _split_excess_waits(nc)
    return nc


def pack_inputs(image, w1, b1, w2, b2, wh, bh):
    """Host-side transforms; returns (shared dict, per-core dict list)."""
    image = np.asarray(image, dtype=np.float32)
    w1 = np.asarray(w1, dtype=np.float32)
    b1 = np.asarray(b1, dtype=np.float32)
    w2 = np.asarray(w2, dtype=np.float32)
    b2 = np.asarray(b2, dtype=np.float32)
    wh = np.asarray(wh, dtype=np.float32)
    bh = np.asarray(bh, dtype=np.float32)

    # Banded stationaries. Partition p = (x - x0)*6 + ci; col m = (x' - m0)*6 + co.
    # value = w[co, ci, ky, x - x' + 2] inside the band; only real output
    # columns (xps) are filled -- pad x' columns stay zero so the x-pad
    # partitions of the slabs are never written with nonzero values.
    def stat(w, cin, cout, x0, xs, xps, m0, M):
        s = np.zeros((KW, 5, M), np.float32)
        for ky in range(5):
            for xi in xs:
                for ci in range(cin):
                    p = (xi - x0) * CH + ci
                    for xp in xps:
                        kx = xi - xp + 2
                        if 0 <= kx < 5:
                            for co in range(cout):
                                m = (xp - m0) * CH + co
                                s[p, ky, m] = w[co, ci, ky, kx]
        return s.reshape(KW, -1)

    # A: inputs x in [-2,18), real outputs x' in [0,16), col origin -2
    w1sa = stat(w1, 5, 6, -2, range(-2, 18), range(0, 16), -2, MA)
    w2sa = stat(w2, 6, 5, -2, range(-2, 18), range(0, 16), -2, MA)
    # B: inputs x in [14,34), real outputs x' in [16,32), col origin 14
    w1sb = stat(w1, 5, 6, 14, range(14, 34), range(16, 32), 14, MB)
    w2sb = stat(w2, 6, 5, 14, range(14, 34), range(16, 32), 14, MB)

    # head stationaries: wha[p=(x+2)*6+ci, y*10+k] = wh[k, ci, y, x]
    wha = np.zeros((KW, 32, 10), np.float32)
    whb = np.zeros((KW, 32, 10), np.float32)
    for x in range(16):
        for ci in range(5):
            wha[(x + 2) * CH + ci] = wh[:, ci, :, x].T  # (y, k)
            whb[(x + 2) * CH + ci] = wh[:, ci, :, x + 16].T
    wha = wha.reshape(KW, 320)
    whb = whb.reshape(KW, 320)

    biasm = np.zeros((128, 8), np.float32)
    for xp in range(16):
        for co in range(6):
            biasm[(xp + 2) * CH + co, 0] = b1[co]
            biasm[(xp + 2) * CH + co, 1] = b1[co]
        for co in range(5):
            biasm[(xp + 2) * CH + co, 2] = b2[co]
            biasm[(xp + 2) * CH + co, 3] = b2[co]
    biasm[0:10, 4] = bh

    shared = {
        "w1sa": w1sa.astype(np.float16), "w1sb": w1sb.astype(np.float16),
        "w2sa": w2sa.astype(np.float16), "w2sb": w2sb.astype(np.float16),
        "wha": wha.astype(np.float16), "whb": whb.astype(np.float16),
        "bias": biasm,
    }

    # c1 = conv(img; w1[:, 5:8]) per core, packed into psum1 (x',co) layout.
    wimg = w1[:, 5:8]  # [6, 3, 5, 5]
    percore = []
    for c in range(NCORES):
        sh = image[c * NPER : (c + 1) * NPER]  # [64, 3, 32, 32]
        xp_ = np.zeros((NPER, 3, 36, 36), np.float32)
        xp_[:, :, 2:34, 2:34] = sh
        out = np.zeros((6, NPER, 32, 32), np.float32)
        for ky in range(5):
            for kx in range(5):
                out += np.tensordot(
                    wimg[:, :, ky, kx], xp_[:, :, ky : ky + 32, kx : kx + 32],
                    axes=([1], [1]),
                )
        # out[co, n, y, x']
        c1a = np.zeros((MA, YP, NPER), np.float32)
        c1b = np.zeros((MB, YP, NPER), np.float32)
        for x in range(16):
            for co in range(6):
                c1a[(x + 2) * CH + co, 2:34, :] = out[co, :, :, x].T
                c1b[(x + 2) * CH + co, 2:34, :] = out[co, :, :, x + 16].T
        percore.append({
            "c1a": c1a.reshape(MA, FREE).astype(np.float16),
            "c1b": c1b.reshape(MB, FREE).astype(np.float16),
        })
    return shared, percore


def make_in_maps(inputs):
    shared, percore = pack_inputs(
        inputs["image"], inputs["w1"], inputs["b1"], inputs["w2"], inputs["b2"],
        inputs["wh"], inputs["bh"],
    )
    return [dict(shared, **percore[c]) for c in range(NCORES)]


_NC_CACHE = {}


def _get_nc(iters, unroll=4):
    key = (iters, unroll)
    if key not in _NC_CACHE:
        _NC_CACHE[key] = build_nc(iters, unroll)
    return _NC_CACHE[key]


def kernel(image, w1, b1, w2, b2, wh, bh, _iters=ITERS, _unroll=4):
    from concourse.bass_utils import run_bass_kernel_spmd

    shared, percore = pack_inputs(image, w1, b1, w2, b2, wh, bh)
    in_maps = [dict(shared, **percore[c]) for c in range(NCORES)]
    nc = _get_nc(_iters, _unroll)
    res = run_bass_kernel_spmd(nc, in_maps, list(range(NCORES)))
    outs = []
    for c in range(NCORES):
        o = res.results[c]["out"]  # [10, 64]
        outs.append(o.T)  # [64, 10]
    logits = np.concatenate(outs, axis=0).astype(np.float32)  # [512, 10]
    return logits.reshape(NTOT, 10, 1, 1)


# revision 10
# speedup vs baseline: 1.1085x; 1.1085x over previous
"""Trainium2 Bass kernel for nn_Classifier_6717328851414 (dense x-major, y-half pipelined, DVE c1-inject).

DEQ-style classifier:
  K iterations of  z <- 0.5*z + 0.5*lrelu(conv2(lrelu(conv1(cat(z, img)))))
  conv1: 8->6 ch 5x5 pad 2; conv2: 6->5 ch 5x5 pad 2; 32x32 images; then a
  5->10 channel 32x32 valid "head" conv producing logits (N,10,1,1).

The fixed point z* = f(z*) is independent of the damping factor, so the
kernel iterates UNDAMPED (alpha=1: z <- f(z)), which contracts at ~0.6/iter
vs the reference's 0.78 (alpha=0.5). ITERS=13 reaches 2.0e-3 of the 150-iter
reference at ~2e-3; ITERS=12 lands at ~3.1e-3 (tolerance 2e-2, ~6.5x
margin), and the z-update reduces to the ScalarE activation writing the z
slab directly (no DVE damping op). Iteration 1 is specialized: z0 = 0, so
conv1's z-matmuls are skipped and h1 = Lrelu(c1 + b1) reads the precomputed
image term directly.

Strategy: pure data parallel over batch N=512 -> 64 images per core.

Per-core layout (fp16 activations, fp32 PSUM):
  Channels padded to 6 slots (z: 5 real + 1 zero; h1: 6 real).
  Two x-chunks with 4-column overlap (halo), partition p = (x - x0)*6 + c:
    slab A: x in [-2, 18)   (x0=-2,  120 partitions used of 128)
    slab B: x in [14, 34)   (x0=14,  120 partitions)
  so the same x maps to pA = pB + 96 (32-aligned partition shift -> legal
  cross-partition DVE halo copies).
  Free dim: (y_padded, n) = 36*64 = 2304; y rows 0,1,34,35 stay zero.

Each conv chunk is ONE dense-K matmul group: K = full 120-partition x-window,
M = (x',co) output columns (108 for A, 120 for B), accumulated over the 5 ky
taps by shifting the moving AP along y, one matmul per (quarter, ky).
Bias + leaky-relu + the (undamped) z-update are ONE ScalarE activation
(Lrelu, alpha=0.01) writing the slab directly. The constant image
contribution to conv1 (c1 = conv(img) over channels 5:8, precomputed on the
host) is injected by a DVE scalar_tensor_tensor (PSUM read -> SBUF staging;
the DVE has no PSUM write port) instead of a PE identity matmul, and kernel
taps whose input rows are all padding are clipped (start=True rides the
always-full ky=2 tap), leaving 80 matmuls / ~39.4k streamed columns per
iteration (~17 us/iter vs the banded baseline's ~34.7 us/iter).
"""

import numpy as np

import concourse.bass as bass
import concourse.mybir as mybir
import concourse.tile as tile
from concourse.vector_clock import ScopedClock, VectorClock

ITERS = 12
SLOPE = 0.01
NCORES = 8
NTOT = 512
NPER = NTOT // NCORES  # 64
YP = 36
FREE = YP * NPER  # 2304
CH = 6
MA = 108  # chunk A output cols: x' in [-2,16)
MB = 120  # chunk B output cols: x' in [14,34)
KW = 120  # moving K: 20 x-cols * 6
F32 = mybir.dt.float32
F16 = mybir.dt.float16
AF = mybir.ActivationFunctionType
OP = mybir.AluOpType


def _patched_drain_and_barrier(self, tick_clock, wait_clock):
    # Workaround: walrus rejects >2 sync waits on one instruction; split the
    # final drain's waits across one SP nop per logical processor.
    gc = tick_clock.global_clock
    n = len(gc)
    for p in range(n):
        if gc[p] == 0:
            continue
        vc = VectorClock([gc[q] if q == p else 0 for q in range(n)])
        nop = self.nc.sync.nop(nofuse=True)
        wait_clock.add_sem_waits(nop.ins, ScopedClock({None: vc}))
    self.nc.sync.drain()
    self.nc.all_engine_barrier()
    assert self.sems is not None
    popped = self.nc._tile_sem_poison_stack.pop()
    assert popped is self._sem_poison
    self.nc.clear_and_free_semaphores(list(self.sems.allocated().values()))
    self.nc.all_engine_barrier()


tile.TileContext._drain_and_barrier = _patched_drain_and_barrier


def _split_excess_waits(nc, limit=1):
    """Hoist excess sync waits onto same-engine NoOps (walrus limit)."""
    for bb in nc.main_func.blocks:
        out = []
        changed = False
        for ins in bb.instructions:
            lim = limit
            si = ins.sync_info
            waits = list(si.on_wait) if (si is not None and si.on_wait) else []
            if len(waits) > lim:
                extra, keep = waits[:-lim], waits[-lim:]
                for i0 in range(0, len(extra), limit):
                    nop = mybir.InstNoOp(
                        name=nc.get_next_instruction_name(),
                        engine=ins.engine,
                        ins=[],
                        outs=[],
                        sync_info=mybir.SyncInfo(
                            on_wait=extra[i0 : i0 + limit], on_update=[]
                        ),
                    )
                    out.append(nop)
                si.on_wait = keep
                changed = True
            out.append(ins)
        if changed:
            bb.instructions = out


def build_nc(iters=ITERS, unroll=4):
    nc = bass.Bass()

    w1sa_p = nc.declare_dram_parameter("w1sa", [KW, 5 * MA], F16, isOutput=False)
    w1sb_p = nc.declare_dram_parameter("w1sb", [KW, 5 * MB], F16, isOutput=False)
    w2sa_p = nc.declare_dram_parameter("w2sa", [KW, 5 * MA], F16, isOutput=False)
    w2sb_p = nc.declare_dram_parameter("w2sb", [KW, 5 * MB], F16, isOutput=False)
    wha_p = nc.declare_dram_parameter("wha", [KW, 320], F16, isOutput=False)
    whb_p = nc.declare_dram_parameter("whb", [KW, 320], F16, isOutput=False)
    bias_p = nc.declare_dram_parameter("bias", [128, 8], F32, isOutput=False)
    c1a_p = nc.declare_dram_parameter("c1a", [MA, FREE], F16, isOutput=False)
    c1b_p = nc.declare_dram_parameter("c1b", [MB, FREE], F16, isOutput=False)
    out_p = nc.declare_dram_parameter("out", [10, NPER], F32, isOutput=True)

    with tile.TileContext(nc) as tc:
        with (
            tc.tile_pool(name="const", bufs=1) as cpool,
            tc.tile_pool(name="state", bufs=1) as spool,
            tc.tile_pool(name="psum", bufs=8, space="PSUM") as ppool,
            tc.tile_pool(name="stage", bufs=4) as vpool,
        ):
            w1sa = cpool.tile([KW, 5 * MA], F16, tag="w1sa")
            w1sb = cpool.tile([KW, 5 * MB], F16, tag="w1sb")
            w2sa = cpool.tile([KW, 5 * MA], F16, tag="w2sa")
            w2sb = cpool.tile([KW, 5 * MB], F16, tag="w2sb")
            wha = cpool.tile([KW, 320], F16, tag="wha")
            whb = cpool.tile([KW, 320], F16, tag="whb")
            bias = cpool.tile([128, 8], F32, tag="bias")
            c1a = cpool.tile([MA, FREE], F16, tag="c1a")
            c1b = cpool.tile([MB, FREE], F16, tag="c1b")
            for t, p in (
                (w1sa, w1sa_p), (w1sb, w1sb_p), (w2sa, w2sa_p), (w2sb, w2sb_p),
                (wha, wha_p), (whb, whb_p), (bias, bias_p),
                (c1a, c1a_p), (c1b, c1b_p),
            ):
                nc.sync.dma_start(t[:], p[:])

            # y-half-split state: half 0 = y in [-2,18) (rows y+2), half 1 =
            # y in [14,34) (rows y-14); 20 padded rows each, 4-row overlap.
            # Quarters 0,1 read/write half 0; quarters 2,3 half 1 -- a
            # quarter's 5-tap y-window never crosses its half. Halo closes
            # for half 0 are issued between the q2 and q3 matmul groups (they
            # depend only on q0..q2), so the next stage's q0/q1 matmuls are
            # ready the moment the PE finishes q3 -- no stage-transition
            # stalls.
            HF = 20 * NPER
            zs = {k: spool.tile([128, HF], F16, tag=f"z{k}", name=f"z{k}")
                  for k in ("A0", "A1", "B0", "B1")}
            h1s = {k: spool.tile([128, HF], F16, tag=f"h1{k}", name=f"h1{k}")
                   for k in ("A0", "A1", "B0", "B1")}
            for t in list(zs.values()) + list(h1s.values()):
                nc.gpsimd.memset(t[:], 0.0)
            R = NPER

            def half0_close(ts):
                # Close half 0 of all four slabs using only q0..q2 content.
                # Regions copied while transiently stale are re-fixed by a
                # later step (s5 fixes A0 rows 18,19; s6a/s6b fix B0's).
                nc.vector.tensor_copy(ts["A1"][:, 0 : 2 * R], ts["A0"][:, 16 * R : 18 * R])          # s1
                nc.vector.tensor_copy(ts["B0"][0:12, :], ts["A0"][96:108, :])                        # s2
                nc.vector.tensor_copy(ts["A0"][:, 18 * R : 20 * R], ts["A1"][:, 2 * R : 4 * R])      # s5
                nc.vector.tensor_copy(ts["B0"][:, 18 * R : 20 * R], ts["B1"][:, 2 * R : 4 * R])       # s6b (partitions 0:12 transiently stale)
                nc.vector.tensor_copy(ts["B0"][0:12, 18 * R : 20 * R], ts["A1"][96:108, 2 * R : 4 * R])  # s6a fixes them
                nc.vector.tensor_copy(ts["A0"][96:128, :], ts["B0"][0:32, :])                        # s7

            def half1_close(ts):
                nc.vector.tensor_copy(ts["B1"][0:12, :], ts["A1"][96:108, :])                        # s3
                nc.vector.tensor_copy(ts["B1"][:, 0 : 2 * R], ts["B0"][:, 16 * R : 18 * R])          # s4
                nc.vector.tensor_copy(ts["A1"][96:128, :], ts["B1"][0:32, :])                        # s8

            def jrng(q, ky):
                # output rows j in [0,8) whose input row 8q+j+ky-2 is real;
                # clipped taps only drop all-zero pad-row contributions.
                return max(0, 2 - ky - 8 * q), min(8, 34 - ky - 8 * q)

            KYS = (2, 0, 1, 3, 4)  # full-coverage tap first (carries start=True)

            def conv1_group(q):
                h = "01"[q // 2]
                r = 8 * (q % 2)
                for M, c1t, w1t, bcol, ab in (
                    (MB, c1b, w1sb, 1, "B"),
                    (MA, c1a, w1sa, 0, "A"),
                ):
                    zt, h1t = zs[ab + h], h1s[ab + h]
                    ps = ppool.tile([M, 512], F32, tag="ps")
                    for i, ky in enumerate(KYS):
                        jlo, jhi = jrng(q, ky)
                        nc.tensor.matmul(
                            ps[:, jlo * R : jhi * R],
                            w1t[:, ky * M : (ky + 1) * M],
                            zt[0:KW, (r + ky + jlo) * R : (r + ky + jhi) * R],
                            start=(i == 0), stop=(i == 4),
                        )
                    co = (8 * q + 2) * R
                    # inject the constant image term on the DVE (PSUM read,
                    # SBUF write -- the DVE has no PSUM write port), then
                    # bias+lrelu on ScalarE from SBUF.
                    u = vpool.tile([M, 512], F16, tag="u")
                    nc.vector.scalar_tensor_tensor(
                        u[:], ps[:], 1.0, c1t[:, co : co + 512], OP.mult, OP.add
                    )
                    nc.scalar.activation(
                        h1t[0:M, (r + 2) * R : (r + 2) * R + 512], u[:], AF.Lrelu,
                        bias=bias[0:M, bcol : bcol + 1], scale=1.0, alpha=SLOPE,
                    )

            def conv2_group(q):
                h = "01"[q // 2]
                r = 8 * (q % 2)
                for M, w2t, bcol, ab in (
                    (MB, w2sb, 3, "B"),
                    (MA, w2sa, 2, "A"),
                ):
                    h1t, zt = h1s[ab + h], zs[ab + h]
                    ps = ppool.tile([M, 512], F32, tag="ps")
                    for i, ky in enumerate(KYS):
                        jlo, jhi = jrng(q, ky)
                        nc.tensor.matmul(
                            ps[:, jlo * R : jhi * R],
                            w2t[:, ky * M : (ky + 1) * M],
                            h1t[0:KW, (r + ky + jlo) * R : (r + ky + jhi) * R],
                            start=(i == 0), stop=(i == 4),
                        )
                    nc.scalar.activation(
                        zt[0:M, (r + 2) * R : (r + 2) * R + 512], ps[:], AF.Lrelu,
                        bias=bias[0:M, bcol : bcol + 1], scale=1.0, alpha=SLOPE,
                    )

            def conv1_first(q):
                # iteration 1: z == 0, so conv1 is just lrelu(c1 + b1)
                h = "01"[q // 2]
                r = 8 * (q % 2)
                for M, c1t, bcol, ab in ((MB, c1b, 1, "B"), (MA, c1a, 0, "A")):
                    nc.scalar.activation(
                        h1s[ab + h][0:M, (r + 2) * R : (r + 2) * R + 512],
                        c1t[:, (8 * q + 2) * R : (8 * q + 2) * R + 512], AF.Lrelu,
                        bias=bias[0:M, bcol : bcol + 1], scale=1.0, alpha=SLOPE,
                    )

            def one_iter(conv1g=None):
                conv1g = conv1g or conv1_group
                for q in (0, 1, 2):
                    conv1g(q)
                half0_close(h1s)
                conv1g(3)
                half1_close(h1s)
                for q in (0, 1, 2):
                    conv2_group(q)
                half0_close(zs)
                conv2_group(3)
                half1_close(zs)

            if iters > 0:
                one_iter(conv1_first)
            iters = max(0, iters - 1)
            trips, rem = divmod(iters, unroll)
            if trips > 0:
                with tc.For_i(0, trips, 1):
                    for _ in range(unroll):
                        one_iter()
            for _ in range(rem):
                one_iter()

            # ---- head: logits[k, n] = sum_{c,y,x} wh * z + bh
            psh = ppool.tile([10, NPER], F32, tag="ps")
            for y in range(32):
                h = "0" if y < 16 else "1"
                off = ((y + 2) if y < 16 else (y - 14)) * NPER
                nc.tensor.matmul(
                    psh[:], wha[:, y * 10 : (y + 1) * 10],
                    zs["A" + h][0:KW, off : off + NPER],
                    start=(y == 0), stop=False,
                )
                nc.tensor.matmul(
                    psh[:], whb[:, y * 10 : (y + 1) * 10],
                    zs["B" + h][0:KW, off : off + NPER],
                    start=False, stop=(y == 31),
                )
            out_sb = vpool.tile([10, NPER], F32, tag="osb")
            nc.scalar.activation(
                out_sb[:], psh[:], AF.Identity, bias=bias[0:10, 4:5], scale=1.0
            )
            nc.sync.dma_start(out_p[:], out_sb[:])

    _split_excess_waits(nc)
    return nc


def pack_inputs(image, w1, b1, w2, b2, wh, bh):
    """Host-side transforms; returns (shared dict, per-core dict list)."""
    image = np.asarray(image, dtype=np.float32)
    w1 = np.asarray(w1, dtype=np.float32)
    b1 = np.asarray(b1, dtype=np.float32)
    w2 = np.asarray(w2, dtype=np.float32)
    b2 = np.asarray(b2, dtype=np.float32)
    wh = np.asarray(wh, dtype=np.float32)
    bh = np.asarray(bh, dtype=np.float32)

    # Banded stationaries. Partition p = (x - x0)*6 + ci; col m = (x' - m0)*6 + co.
    # value = w[co, ci, ky, x - x' + 2] inside the band; only real output
    # columns (xps) are filled -- pad x' columns stay zero so the x-pad
    # partitions of the slabs are never written with nonzero values.
    def stat(w, cin, cout, x0, xs, xps, m0, M):
        s = np.zeros((KW, 5, M), np.float32)
        for ky in range(5):
            for xi in xs:
                for ci in range(cin):
                    p = (xi - x0) * CH + ci
                    for xp in xps:
                        kx = xi - xp + 2
                        if 0 <= kx < 5:
                            for co in range(cout):
                                m = (xp - m0) * CH + co
                                s[p, ky, m] = w[co, ci, ky, kx]
        return s.reshape(KW, -1)

    # A: inputs x in [-2,18), real outputs x' in [0,16), col origin -2
    w1sa = stat(w1, 5, 6, -2, range(-2, 18), range(0, 16), -2, MA)
    w2sa = stat(w2, 6, 5, -2, range(-2, 18), range(0, 16), -2, MA)
    # B: inputs x in [14,34), real outputs x' in [16,32), col origin 14
    w1sb = stat(w1, 5, 6, 14, range(14, 34), range(16, 32), 14, MB)
    w2sb = stat(w2, 6, 5, 14, range(14, 34), range(16, 32), 14, MB)

    # head stationaries: wha[p=(x+2)*6+ci, y*10+k] = wh[k, ci, y, x]
    wha = np.zeros((KW, 32, 10), np.float32)
    whb = np.zeros((KW, 32, 10), np.float32)
    for x in range(16):
        for ci in range(5):
            wha[(x + 2) * CH + ci] = wh[:, ci, :, x].T  # (y, k)
            whb[(x + 2) * CH + ci] = wh[:, ci, :, x + 16].T
    wha = wha.reshape(KW, 320)
    whb = whb.reshape(KW, 320)

    biasm = np.zeros((128, 8), np.float32)
    for xp in range(16):
        for co in range(6):
            biasm[(xp + 2) * CH + co, 0] = b1[co]
            biasm[(xp + 2) * CH + co, 1] = b1[co]
        for co in range(5):
            biasm[(xp + 2) * CH + co, 2] = b2[co]
            biasm[(xp + 2) * CH + co, 3] = b2[co]
    biasm[0:10, 4] = bh

    shared = {
        "w1sa": w1sa.astype(np.float16), "w1sb": w1sb.astype(np.float16),
        "w2sa": w2sa.astype(np.float16), "w2sb": w2sb.astype(np.float16),
        "wha": wha.astype(np.float16), "whb": whb.astype(np.float16),
        "bias": biasm,
    }

    # c1 = conv(img; w1[:, 5:8]) per core, packed into psum1 (x',co) layout.
    wimg = w1[:, 5:8]  # [6, 3, 5, 5]
    percore = []
    for c in range(NCORES):
        sh = image[c * NPER : (c + 1) * NPER]  # [64, 3, 32, 32]
        xp_ = np.zeros((NPER, 3, 36, 36), np.float32)
        xp_[:, :, 2:34, 2:34] = sh
        out = np.zeros((6, NPER, 32, 32), np.float32)
        for ky in range(5):
            for kx in range(5):
                out += np.tensordot(
                    wimg[:, :, ky, kx], xp_[:, :, ky : ky + 32, kx : kx + 32],
                    axes=([1], [1]),
                )
        # out[co, n, y, x']
        c1a = np.zeros((MA, YP, NPER), np.float32)
        c1b = np.zeros((MB, YP, NPER), np.float32)
        for x in range(16):
            for co in range(6):
                c1a[(x + 2) * CH + co, 2:34, :] = out[co, :, :, x].T
                c1b[(x + 2) * CH + co, 2:34, :] = out[co, :, :, x + 16].T
        percore.append({
            "c1a": c1a.reshape(MA, FREE).astype(np.float16),
            "c1b": c1b.reshape(MB, FREE).astype(np.float16),
        })
    return shared, percore


def make_in_maps(inputs):
    shared, percore = pack_inputs(
        inputs["image"], inputs["w1"], inputs["b1"], inputs["w2"], inputs["b2"],
        inputs["wh"], inputs["bh"],
    )
    return [dict(shared, **percore[c]) for c in range(NCORES)]


_NC_CACHE = {}


def _get_nc(iters, unroll=4):
    key = (iters, unroll)
    if key not in _NC_CACHE:
        _NC_CACHE[key] = build_nc(iters, unroll)
    return _NC_CACHE[key]


def kernel(image, w1, b1, w2, b2, wh, bh, _iters=ITERS, _unroll=4):
    from concourse.bass_utils import run_bass_kernel_spmd

    shared, percore = pack_inputs(image, w1, b1, w2, b2, wh, bh)
    in_maps = [dict(shared, **percore[c]) for c in range(NCORES)]
    nc = _get_nc(_iters, _unroll)
    res = run_bass_kernel_spmd(nc, in_maps, list(range(NCORES)))
    outs = []
    for c in range(NCORES):
        o = res.results[c]["out"]  # [10, 64]
        outs.append(o.T)  # [64, 10]
    logits = np.concatenate(outs, axis=0).astype(np.float32)  # [512, 10]
    return logits.reshape(NTOT, 10, 1, 1)


# revision 11
# speedup vs baseline: 4.5391x; 4.0950x over previous
"""Trainium2 Bass kernel for nn_Classifier_6717328851414 (dense x-major, y-half pipelined, DVE c1-inject).

DEQ-style classifier:
  K iterations of  z <- 0.5*z + 0.5*lrelu(conv2(lrelu(conv1(cat(z, img)))))
  conv1: 8->6 ch 5x5 pad 2; conv2: 6->5 ch 5x5 pad 2; 32x32 images; then a
  5->10 channel 32x32 valid "head" conv producing logits (N,10,1,1).

The fixed point z* = f(z*) is independent of the damping factor, so the
kernel iterates UNDAMPED (alpha=1: z <- f(z)), which contracts at ~0.6/iter
vs the reference's 0.78 (alpha=0.5). ITERS=13 reaches 2.0e-3 of the 150-iter
reference at ~2e-3; ITERS=11 lands at ~5.3e-3 (tolerance 2e-2, ~3.8x
margin; 12 iters -> 3.1e-3 if more margin is wanted), and the z-update reduces to the ScalarE activation writing the z
slab directly (no DVE damping op). Iteration 1 is specialized: z0 = 0, so
conv1's z-matmuls are skipped and h1 = Lrelu(c1 + b1) reads the precomputed
image term directly.

Strategy: pure data parallel over batch N=512 -> 64 images per core.

Per-core layout (fp16 activations, fp32 PSUM):
  Channels padded to 6 slots (z: 5 real + 1 zero; h1: 6 real).
  Two x-chunks with 4-column overlap (halo), partition p = (x - x0)*6 + c:
    slab A: x in [-2, 18)   (x0=-2,  120 partitions used of 128)
    slab B: x in [14, 34)   (x0=14,  120 partitions)
  so the same x maps to pA = pB + 96 (32-aligned partition shift -> legal
  cross-partition DVE halo copies).
  Free dim: (y_padded, n) = 36*64 = 2304; y rows 0,1,34,35 stay zero.

Each conv chunk is ONE dense-K matmul group: K = full 120-partition x-window,
M = (x',co) output columns (108 for A, 120 for B), accumulated over the 5 ky
taps by shifting the moving AP along y, one matmul per (quarter, ky).
Bias + leaky-relu + the (undamped) z-update are ONE ScalarE activation
(Lrelu, alpha=0.01) writing the slab directly. The constant image
contribution to conv1 (c1 = conv(img) over channels 5:8, precomputed on the
host) is injected by a DVE scalar_tensor_tensor (PSUM read -> SBUF staging;
the DVE has no PSUM write port) instead of a PE identity matmul, and kernel
taps whose input rows are all padding are clipped (start=True rides the
always-full ky=2 tap), leaving 80 matmuls / ~39.4k streamed columns per
iteration (~17 us/iter vs the banded baseline's ~34.7 us/iter).
"""

import numpy as np

import concourse.bass as bass
import concourse.mybir as mybir
import concourse.tile as tile
from concourse.vector_clock import ScopedClock, VectorClock

ITERS = 11
SLOPE = 0.01
NCORES = 8
NTOT = 512
NPER = NTOT // NCORES  # 64
YP = 36
FREE = YP * NPER  # 2304
CH = 6
MA = 108  # chunk A output cols: x' in [-2,16)
MB = 120  # chunk B output cols: x' in [14,34)
KW = 120  # moving K: 20 x-cols * 6
F32 = mybir.dt.float32
F16 = mybir.dt.float16
AF = mybir.ActivationFunctionType
OP = mybir.AluOpType


def _patched_drain_and_barrier(self, tick_clock, wait_clock):
    # Workaround: walrus rejects >2 sync waits on one instruction; split the
    # final drain's waits across one SP nop per logical processor.
    gc = tick_clock.global_clock
    n = len(gc)
    for p in range(n):
        if gc[p] == 0:
            continue
        vc = VectorClock([gc[q] if q == p else 0 for q in range(n)])
        nop = self.nc.sync.nop(nofuse=True)
        wait_clock.add_sem_waits(nop.ins, ScopedClock({None: vc}))
    self.nc.sync.drain()
    self.nc.all_engine_barrier()
    assert self.sems is not None
    popped = self.nc._tile_sem_poison_stack.pop()
    assert popped is self._sem_poison
    self.nc.clear_and_free_semaphores(list(self.sems.allocated().values()))
    self.nc.all_engine_barrier()


tile.TileContext._drain_and_barrier = _patched_drain_and_barrier


def _split_excess_waits(nc, limit=1):
    """Hoist excess sync waits onto same-engine NoOps (walrus limit)."""
    for bb in nc.main_func.blocks:
        out = []
        changed = False
        for ins in bb.instructions:
            lim = limit
            si = ins.sync_info
            waits = list(si.on_wait) if (si is not None and si.on_wait) else []
            if len(waits) > lim:
                extra, keep = waits[:-lim], waits[-lim:]
                for i0 in range(0, len(extra), limit):
                    nop = mybir.InstNoOp(
                        name=nc.get_next_instruction_name(),
                        engine=ins.engine,
                        ins=[],
                        outs=[],
                        sync_info=mybir.SyncInfo(
                            on_wait=extra[i0 : i0 + limit], on_update=[]
                        ),
                    )
                    out.append(nop)
                si.on_wait = keep
                changed = True
            out.append(ins)
        if changed:
            bb.instructions = out


def build_nc(iters=ITERS, unroll=4):
    nc = bass.Bass()

    w1sa_p = nc.declare_dram_parameter("w1sa", [KW, 5 * MA], F16, isOutput=False)
    w1sb_p = nc.declare_dram_parameter("w1sb", [KW, 5 * MB], F16, isOutput=False)
    w2sa_p = nc.declare_dram_parameter("w2sa", [KW, 5 * MA], F16, isOutput=False)
    w2sb_p = nc.declare_dram_parameter("w2sb", [KW, 5 * MB], F16, isOutput=False)
    wha_p = nc.declare_dram_parameter("wha", [KW, 320], F16, isOutput=False)
    whb_p = nc.declare_dram_parameter("whb", [KW, 320], F16, isOutput=False)
    bias_p = nc.declare_dram_parameter("bias", [128, 8], F32, isOutput=False)
    c1a_p = nc.declare_dram_parameter("c1a", [MA, FREE], F16, isOutput=False)
    c1b_p = nc.declare_dram_parameter("c1b", [MB, FREE], F16, isOutput=False)
    out_p = nc.declare_dram_parameter("out", [10, NPER], F32, isOutput=True)

    with tile.TileContext(nc) as tc:
        with (
            tc.tile_pool(name="const", bufs=1) as cpool,
            tc.tile_pool(name="state", bufs=1) as spool,
            tc.tile_pool(name="psum", bufs=8, space="PSUM") as ppool,
            tc.tile_pool(name="stage", bufs=4) as vpool,
        ):
            w1sa = cpool.tile([KW, 5 * MA], F16, tag="w1sa")
            w1sb = cpool.tile([KW, 5 * MB], F16, tag="w1sb")
            w2sa = cpool.tile([KW, 5 * MA], F16, tag="w2sa")
            w2sb = cpool.tile([KW, 5 * MB], F16, tag="w2sb")
            wha = cpool.tile([KW, 320], F16, tag="wha")
            whb = cpool.tile([KW, 320], F16, tag="whb")
            bias = cpool.tile([128, 8], F32, tag="bias")
            c1a = cpool.tile([MA, FREE], F16, tag="c1a")
            c1b = cpool.tile([MB, FREE], F16, tag="c1b")
            for t, p in (
                (w1sa, w1sa_p), (w1sb, w1sb_p), (w2sa, w2sa_p), (w2sb, w2sb_p),
                (wha, wha_p), (whb, whb_p), (bias, bias_p),
                (c1a, c1a_p), (c1b, c1b_p),
            ):
                nc.sync.dma_start(t[:], p[:])

            # y-half-split state: half 0 = y in [-2,18) (rows y+2), half 1 =
            # y in [14,34) (rows y-14); 20 padded rows each, 4-row overlap.
            # Quarters 0,1 read/write half 0; quarters 2,3 half 1 -- a
            # quarter's 5-tap y-window never crosses its half. Halo closes
            # for half 0 are issued between the q2 and q3 matmul groups (they
            # depend only on q0..q2), so the next stage's q0/q1 matmuls are
            # ready the moment the PE finishes q3 -- no stage-transition
            # stalls.
            HF = 20 * NPER
            zs = {k: spool.tile([128, HF], F16, tag=f"z{k}", name=f"z{k}")
                  for k in ("A0", "A1", "B0", "B1")}
            h1s = {k: spool.tile([128, HF], F16, tag=f"h1{k}", name=f"h1{k}")
                   for k in ("A0", "A1", "B0", "B1")}
            for t in list(zs.values()) + list(h1s.values()):
                nc.gpsimd.memset(t[:], 0.0)
            R = NPER

            def half0_close(ts):
                # Close half 0 of all four slabs using only q0..q2 content.
                # Regions copied while transiently stale are re-fixed by a
                # later step (s5 fixes A0 rows 18,19; s6a/s6b fix B0's).
                nc.vector.tensor_copy(ts["A1"][:, 0 : 2 * R], ts["A0"][:, 16 * R : 18 * R])          # s1
                nc.vector.tensor_copy(ts["B0"][0:12, :], ts["A0"][96:108, :])                        # s2
                nc.vector.tensor_copy(ts["A0"][:, 18 * R : 20 * R], ts["A1"][:, 2 * R : 4 * R])      # s5
                nc.vector.tensor_copy(ts["B0"][:, 18 * R : 20 * R], ts["B1"][:, 2 * R : 4 * R])       # s6b (partitions 0:12 transiently stale)
                nc.vector.tensor_copy(ts["B0"][0:12, 18 * R : 20 * R], ts["A1"][96:108, 2 * R : 4 * R])  # s6a fixes them
                nc.vector.tensor_copy(ts["A0"][96:128, :], ts["B0"][0:32, :])                        # s7

            def half1_close(ts):
                nc.vector.tensor_copy(ts["B1"][0:12, :], ts["A1"][96:108, :])                        # s3
                nc.vector.tensor_copy(ts["B1"][:, 0 : 2 * R], ts["B0"][:, 16 * R : 18 * R])          # s4
                nc.vector.tensor_copy(ts["A1"][96:128, :], ts["B1"][0:32, :])                        # s8

            def jrng(q, ky):
                # output rows j in [0,8) whose input row 8q+j+ky-2 is real;
                # clipped taps only drop all-zero pad-row contributions.
                return max(0, 2 - ky - 8 * q), min(8, 34 - ky - 8 * q)

            KYS = (2, 0, 1, 3, 4)  # full-coverage tap first (carries start=True)

            def conv1_group(q):
                h = "01"[q // 2]
                r = 8 * (q % 2)
                for M, c1t, w1t, bcol, ab in (
                    (MB, c1b, w1sb, 1, "B"),
                    (MA, c1a, w1sa, 0, "A"),
                ):
                    zt, h1t = zs[ab + h], h1s[ab + h]
                    ps = ppool.tile([M, 512], F32, tag="ps")
                    for i, ky in enumerate(KYS):
                        jlo, jhi = jrng(q, ky)
                        nc.tensor.matmul(
                            ps[:, jlo * R : jhi * R],
                            w1t[:, ky * M : (ky + 1) * M],
                            zt[0:KW, (r + ky + jlo) * R : (r + ky + jhi) * R],
                            start=(i == 0), stop=(i == 4),
                        )
                    co = (8 * q + 2) * R
                    # inject the constant image term on the DVE (PSUM read,
                    # SBUF write -- the DVE has no PSUM write port), then
                    # bias+lrelu on ScalarE from SBUF.
                    u = vpool.tile([M, 512], F16, tag="u")
                    nc.vector.scalar_tensor_tensor(
                        u[:], ps[:], 1.0, c1t[:, co : co + 512], OP.mult, OP.add
                    )
                    nc.scalar.activation(
                        h1t[0:M, (r + 2) * R : (r + 2) * R + 512], u[:], AF.Lrelu,
                        bias=bias[0:M, bcol : bcol + 1], scale=1.0, alpha=SLOPE,
                    )

            def conv2_group(q):
                h = "01"[q // 2]
                r = 8 * (q % 2)
                for M, w2t, bcol, ab in (
                    (MB, w2sb, 3, "B"),
                    (MA, w2sa, 2, "A"),
                ):
                    h1t, zt = h1s[ab + h], zs[ab + h]
                    ps = ppool.tile([M, 512], F32, tag="ps")
                    for i, ky in enumerate(KYS):
                        jlo, jhi = jrng(q, ky)
                        nc.tensor.matmul(
                            ps[:, jlo * R : jhi * R],
                            w2t[:, ky * M : (ky + 1) * M],
                            h1t[0:KW, (r + ky + jlo) * R : (r + ky + jhi) * R],
                            start=(i == 0), stop=(i == 4),
                        )
                    nc.scalar.activation(
                        zt[0:M, (r + 2) * R : (r + 2) * R + 512], ps[:], AF.Lrelu,
                        bias=bias[0:M, bcol : bcol + 1], scale=1.0, alpha=SLOPE,
                    )

            def conv1_first(q):
                # iteration 1: z == 0, so conv1 is just lrelu(c1 + b1)
                h = "01"[q // 2]
                r = 8 * (q % 2)
                for M, c1t, bcol, ab in ((MB, c1b, 1, "B"), (MA, c1a, 0, "A")):
                    nc.scalar.activation(
                        h1s[ab + h][0:M, (r + 2) * R : (r + 2) * R + 512],
                        c1t[:, (8 * q + 2) * R : (8 * q + 2) * R + 512], AF.Lrelu,
                        bias=bias[0:M, bcol : bcol + 1], scale=1.0, alpha=SLOPE,
                    )

            def one_iter(conv1g=None):
                conv1g = conv1g or conv1_group
                for q in (0, 1, 2):
                    conv1g(q)
                half0_close(h1s)
                conv1g(3)
                half1_close(h1s)
                for q in (0, 1, 2):
                    conv2_group(q)
                half0_close(zs)
                conv2_group(3)
                half1_close(zs)

            if iters > 0:
                one_iter(conv1_first)
            iters = max(0, iters - 1)
            trips, rem = divmod(iters, unroll)
            if trips > 0:
                with tc.For_i(0, trips, 1):
                    for _ in range(unroll):
                        one_iter()
            for _ in range(rem):
                one_iter()

            # ---- head: logits[k, n] = sum_{c,y,x} wh * z + bh
            psh = ppool.tile([10, NPER], F32, tag="ps")
            for y in range(32):
                h = "0" if y < 16 else "1"
                off = ((y + 2) if y < 16 else (y - 14)) * NPER
                nc.tensor.matmul(
                    psh[:], wha[:, y * 10 : (y + 1) * 10],
                    zs["A" + h][0:KW, off : off + NPER],
                    start=(y == 0), stop=False,
                )
                nc.tensor.matmul(
                    psh[:], whb[:, y * 10 : (y + 1) * 10],
                    zs["B" + h][0:KW, off : off + NPER],
                    start=False, stop=(y == 31),
                )
            out_sb = vpool.tile([10, NPER], F32, tag="osb")
            nc.scalar.activation(
                out_sb[:], psh[:], AF.Identity, bias=bias[0:10, 4:5], scale=1.0
            )
            nc.sync.dma_start(out_p[:], out_sb[:])

    _split_excess_waits(nc)
    return nc


def pack_inputs(image, w1, b1, w2, b2, wh, bh):
    """Host-side transforms; returns (shared dict, per-core dict list)."""
    image = np.asarray(image, dtype=np.float32)
    w1 = np.asarray(w1, dtype=np.float32)
    b1 = np.asarray(b1, dtype=np.float32)
    w2 = np.asarray(w2, dtype=np.float32)
    b2 = np.asarray(b2, dtype=np.float32)
    wh = np.asarray(wh, dtype=np.float32)
    bh = np.asarray(bh, dtype=np.float32)

    # Banded stationaries. Partition p = (x - x0)*6 + ci; col m = (x' - m0)*6 + co.
    # value = w[co, ci, ky, x - x' + 2] inside the band; only real output
    # columns (xps) are filled -- pad x' columns stay zero so the x-pad
    # partitions of the slabs are never written with nonzero values.
    def stat(w, cin, cout, x0, xs, xps, m0, M):
        s = np.zeros((KW, 5, M), np.float32)
        for ky in range(5):
            for xi in xs:
                for ci in range(cin):
                    p = (xi - x0) * CH + ci
                    for xp in xps:
                        kx = xi - xp + 2
                        if 0 <= kx < 5:
                            for co in range(cout):
                                m = (xp - m0) * CH + co
                                s[p, ky, m] = w[co, ci, ky, kx]
        return s.reshape(KW, -1)

    # A: inputs x in [-2,18), real outputs x' in [0,16), col origin -2
    w1sa = stat(w1, 5, 6, -2, range(-2, 18), range(0, 16), -2, MA)
    w2sa = stat(w2, 6, 5, -2, range(-2, 18), range(0, 16), -2, MA)
    # B: inputs x in [14,34), real outputs x' in [16,32), col origin 14
    w1sb = stat(w1, 5, 6, 14, range(14, 34), range(16, 32), 14, MB)
    w2sb = stat(w2, 6, 5, 14, range(14, 34), range(16, 32), 14, MB)

    # head stationaries: wha[p=(x+2)*6+ci, y*10+k] = wh[k, ci, y, x]
    wha = np.zeros((KW, 32, 10), np.float32)
    whb = np.zeros((KW, 32, 10), np.float32)
    for x in range(16):
        for ci in range(5):
            wha[(x + 2) * CH + ci] = wh[:, ci, :, x].T  # (y, k)
            whb[(x + 2) * CH + ci] = wh[:, ci, :, x + 16].T
    wha = wha.reshape(KW, 320)
    whb = whb.reshape(KW, 320)

    biasm = np.zeros((128, 8), np.float32)
    for xp in range(16):
        for co in range(6):
            biasm[(xp + 2) * CH + co, 0] = b1[co]
            biasm[(xp + 2) * CH + co, 1] = b1[co]
        for co in range(5):
            biasm[(xp + 2) * CH + co, 2] = b2[co]
            biasm[(xp + 2) * CH + co, 3] = b2[co]
    biasm[0:10, 4] = bh

    shared = {
        "w1sa": w1sa.astype(np.float16), "w1sb": w1sb.astype(np.float16),
        "w2sa": w2sa.astype(np.float16), "w2sb": w2sb.astype(np.float16),
        "wha": wha.astype(np.float16), "whb": whb.astype(np.float16),
        "bias": biasm,
    }

    # c1 = conv(img; w1[:, 5:8]) per core, packed into psum1 (x',co) layout.
    wimg = w1[:, 5:8]  # [6, 3, 5, 5]
    percore = []
    for c in range(NCORES):
        sh = image[c * NPER : (c + 1) * NPER]  # [64, 3, 32, 32]
        xp_ = np.zeros((NPER, 3, 36, 36), np.float32)
        xp_[:, :, 2:34, 2:34] = sh
        out = np.zeros((6, NPER, 32, 32), np.float32)
        for ky in range(5):
            for kx in range(5):
                out += np.tensordot(
                    wimg[:, :, ky, kx], xp_[:, :, ky : ky + 32, kx : kx + 32],
                    axes=([1], [1]),
                )
        # out[co, n, y, x']
        c1a = np.zeros((MA, YP, NPER), np.float32)
        c1b = np.zeros((MB, YP, NPER), np.float32)
        for x in range(16):
            for co in range(6):
                c1a[(x + 2) * CH + co, 2:34, :] = out[co, :, :, x].T
                c1b[(x + 2) * CH + co, 2:34, :] = out[co, :, :, x + 16].T
        percore.append({
            "c1a": c1a.reshape(MA, FREE).astype(np.float16),
            "c1b": c1b.reshape(MB, FREE).astype(np.float16),
        })
    return shared, percore


def make_in_maps(inputs):
    shared, percore = pack_inputs(
        inputs["image"], inputs["w1"], inputs["b1"], inputs["w2"], inputs["b2"],
        inputs["wh"], inputs["bh"],
    )
    return [dict(shared, **percore[c]) for c in range(NCORES)]


_NC_CACHE = {}


def _get_nc(iters, unroll=4):
    key = (iters, unroll)
    if key not in _NC_CACHE:
        _NC_CACHE[key] = build_nc(iters, unroll)
    return _NC_CACHE[key]


def kernel(image, w1, b1, w2, b2, wh, bh, _iters=ITERS, _unroll=4):
    from concourse.bass_utils import run_bass_kernel_spmd

    shared, percore = pack_inputs(image, w1, b1, w2, b2, wh, bh)
    in_maps = [dict(shared, **percore[c]) for c in range(NCORES)]
    nc = _get_nc(_iters, _unroll)
    res = run_bass_kernel_spmd(nc, in_maps, list(range(NCORES)))
    outs = []
    for c in range(NCORES):
        o = res.results[c]["out"]  # [10, 64]
        outs.append(o.T)  # [64, 10]
    logits = np.concatenate(outs, axis=0).astype(np.float32)  # [512, 10]
    return logits.reshape(NTOT, 10, 1, 1)
